# revision 1
# baseline (speedup 1.0000x reference)
"""Trainium2 Bass kernel for nn_GTN_72679436583060 (GTN message passing).

Math: with w-softmax over a singleton axis each GTConv is exactly 2*A, so

    out = 2 * rownorm(4*A@A + I) @ A
        = diag(8 / (4*rowsum(M) + 1)) @ (M@A + 0.25*A)   with M = A@A

Sharding: row-wise over 8 cores, A replicated. Per core (rows R = 256):
  GEMM1 (transposed):  MT = A^T @ (A_rows^T)        (2048 x 256), lhsT = A tiles
  deg:                 rowsum(M) via a ones-column matmul sharing GEMM2's lhsT
  GEMM2:               P = M @ A + 0.25*A_rows       (256 x 2048), lhsT = MT tiles
  epilogue:            out = P * (8 / (4*deg + 1))   per-row scale

All matmuls in bf16 (1 cycle/row on PE), fp32 PSUM accumulation, fp32 output.
GEMM1 runs k-outer so the PE tracks the streaming A DMA; all 16 output tile
groups fit in 8 PSUM banks via zero-writing "bank clear" matmuls (which also
warm up the PE HAM clock during the initial DMA window).
"""

import numpy as np

N = 2048
P = 128
NCORES = 8
R = N // NCORES        # 256 rows per core
KT = N // P            # 16 partition tiles
MT = R // P            # 2 row subtiles per core
FD = 512               # PSUM bank free dim (fp32)
NT2 = N // FD          # 4 GEMM2 n-tiles

_CACHE = {}


def _build_bass():
    from contextlib import ExitStack

    import concourse.bass as bass  # noqa: F401
    import concourse.mybir as mybir
    import concourse.tile as tile
    from concourse import bacc

    dt = mybir.dt
    fp32 = dt.float32
    bf16 = dt.bfloat16
    Alu = mybir.AluOpType

    nc = bacc.Bacc(None, target_bir_lowering=False)
    a_d = nc.dram_tensor("a", [N, N], bf16, kind="ExternalInput")
    art_d = nc.dram_tensor("art", [N, R], bf16, kind="ExternalInput")
    ar_d = nc.dram_tensor("ar", [R, N], bf16, kind="ExternalInput")
    ones_d = nc.dram_tensor("ones", [P, 1], bf16, kind="ExternalInput")
    iq_d = nc.dram_tensor("iq", [P, P], bf16, kind="ExternalInput")
    out_d = nc.dram_tensor("out", [R, N], fp32, kind="ExternalOutput")

    with tile.TileContext(nc) as tc, ExitStack() as ctx:
        a_pool = ctx.enter_context(tc.tile_pool(name="a", bufs=KT))
        art_pool = ctx.enter_context(tc.tile_pool(name="art", bufs=KT))
        ar_pool = ctx.enter_context(tc.tile_pool(name="ar", bufs=MT))
        mt_pool = ctx.enter_context(tc.tile_pool(name="mt", bufs=KT))
        const_pool = ctx.enter_context(tc.tile_pool(name="const", bufs=1))
        outsb_pool = ctx.enter_context(tc.tile_pool(name="outsb", bufs=4))
        sc_pool = ctx.enter_context(tc.tile_pool(name="sc", bufs=4))

        zeros_t = const_pool.tile([P, FD], bf16, tag="zeros")
        nc.vector.memset(zeros_t[:], 0.0)

        # Stream A row-tiles (and the matching ART tiles) in k order; they
        # stay resident: GEMM1 uses A as lhsT, GEMM2 reuses it as rhs.
        # The tiny const/ar loads are issued last — they are only needed in
        # GEMM2, and issuing them first would delay the first k-sweep.
        # The stream is HBM-bound (~330 GB/s aggregate); alternating the
        # big A tiles between the two HWDGE queues (sync/scalar) with
        # per-tile granularity keeps the k-sweep dependencies thin.
        a_tiles, art_tiles = [], []
        for k in range(KT):
            rt = art_pool.tile([P, R], bf16, tag="art")
            nc.sync.dma_start(rt[:], art_d[k * P:(k + 1) * P, :])
            art_tiles.append(rt)
            at = a_pool.tile([P, N], bf16, tag="a")
            eng = nc.sync if k % 2 == 0 else nc.scalar
            eng.dma_start(at[:], a_d[k * P:(k + 1) * P, :])
            a_tiles.append(at)
        ar_tiles = []
        for m in range(MT):
            t = ar_pool.tile([P, N], bf16, tag="ar")
            nc.sync.dma_start(t[:], ar_d[m * P:(m + 1) * P, :])
            ar_tiles.append(t)
        ones_t = const_pool.tile([P, 1], bf16, tag="ones")
        nc.sync.dma_start(ones_t[:], ones_d[:, :])
        iq_t = const_pool.tile([P, P], bf16, tag="iq")
        nc.sync.dma_start(iq_t[:], iq_d[:, :])

        # ---- GEMM1: MT[j, r] = sum_k A[k, j] * A_rows[r, k], k-outer ----
        # Two j-groups share each PSUM bank. A start=True zero matmul per
        # bank clears it and sets every has_written bit, so all real
        # matmuls accumulate with start=False regardless of issue order.
        mt_tiles = [None] * KT
        # One shared PSUM pool (8 banks, one tag) for GEMM1 pair tiles,
        # GEMM2 output tiles and deg tiles: GEMM2's first allocations reuse
        # slots as soon as individual pair tiles are copied out, instead of
        # stalling on a whole-pool release at the phase boundary.
        with tc.tile_pool(name="psum", bufs=8, space="PSUM") as psum_pool:
            # Per-bank zero matmul: start=True clears the whole bank; writing
            # [255:257) spans both half-bank groups, so WAW deps keep every
            # real matmul ordered after the clear. Elements outside [255:257)
            # keep has_written unset, so each group's first real matmul
            # overwrites (= accumulate onto zero).
            pairs = []
            for b in range(KT // 2):
                ps = psum_pool.tile([P, FD], fp32, tag="bank", name=f"pair_{b}")
                nc.tensor.matmul(
                    ps[:, R - 1:R + 1], zeros_t[:, 0:P], zeros_t[:, 0:2],
                    start=True, stop=False, skip_group_check=True,
                )
                pairs.append(ps)
            for k in range(KT):
                for j in range(KT):
                    half = (j % 2) * R
                    nc.tensor.matmul(
                        pairs[j // 2][:, half:half + R],
                        a_tiles[k][:, j * P:(j + 1) * P],
                        art_tiles[k][:],
                        start=False, stop=(k == KT - 1),
                        skip_group_check=True,
                    )
            for j in range(KT):
                half = (j % 2) * R
                mt = mt_pool.tile([P, R], bf16, tag="mt")
                nc.vector.tensor_copy(mt[:], pairs[j // 2][:, half:half + R])
                mt_tiles[j] = mt

            # ---- GEMM2 + deg + epilogue ----
            # The 0.25*I matmul doubles as each bank's accumulation-group
            # starter (start=True clears the bank and seeds it with
            # 0.25*A_rows), so banks finish at their last j matmul.
            # m=0 runs j-outer (tracks the mt copies with no stall);
            # m=1 runs n-outer so its four banks complete staggered and the
            # final epilogues pipeline with PE instead of bunching at the end.
            def emit_epilogue(m, n, psum_tile, sca):
                ot = outsb_pool.tile([P, FD], fp32, tag="ot",
                                     name=f"ot_{m}_{n}")
                nc.vector.tensor_scalar(
                    out=ot[:], in0=psum_tile[:], scalar1=sca[:],
                    scalar2=None, op0=Alu.mult,
                )
                eng = nc.sync if n % 2 == 0 else nc.scalar
                eng.dma_start(
                    out_d[m * P:(m + 1) * P, n * FD:(n + 1) * FD], ot[:]
                )

            def emit_deg_scale(m, deg_ps):
                # scale = 8 / (4*deg + 1) == 1 / (0.5*deg + 0.125)
                t1 = sc_pool.tile([P, 1], fp32, tag="t1", name=f"t1_{m}")
                nc.vector.tensor_scalar(
                    out=t1[:], in0=deg_ps[:], scalar1=0.5, scalar2=0.125,
                    op0=Alu.mult, op1=Alu.add,
                )
                sca = sc_pool.tile([P, 1], fp32, tag="sca", name=f"sca_{m}")
                nc.vector.reciprocal(sca[:], t1[:])
                return sca

            # m = 0: j-outer
            m = 0
            outs_ps = [psum_pool.tile([P, FD], fp32, tag="bank",
                                      name=f"outps0_{i}") for i in range(NT2)]
            deg_full = psum_pool.tile([P, FD], fp32, tag="bank", name="deg_0")
            deg_ps = deg_full[:, 0:1]
            for n in range(NT2):
                nc.tensor.matmul(
                    outs_ps[n][:], iq_t[:],
                    ar_tiles[m][:, n * FD:(n + 1) * FD],
                    start=True, stop=False,
                )
            for j in range(KT):
                lhsT = mt_tiles[j][:, m * P:(m + 1) * P]
                for n in range(NT2):
                    nc.tensor.matmul(
                        outs_ps[n][:], lhsT,
                        a_tiles[j][:, n * FD:(n + 1) * FD],
                        start=False, stop=(j == KT - 1),
                    )
                nc.tensor.matmul(
                    deg_ps[:], lhsT, ones_t[:],
                    start=(j == 0), stop=(j == KT - 1),
                )
            sca = emit_deg_scale(m, deg_ps)
            for n in range(NT2):
                emit_epilogue(m, n, outs_ps[n], sca)

            # m = 1: n-outer, deg rides along with the n=0 bank
            m = 1
            deg_full = psum_pool.tile([P, FD], fp32, tag="bank", name="deg_1")
            deg_ps = deg_full[:, 0:1]
            sca = None
            for n in range(NT2):
                ops = psum_pool.tile([P, FD], fp32, tag="bank",
                                     name=f"outps1_{n}")
                nc.tensor.matmul(
                    ops[:], iq_t[:], ar_tiles[m][:, n * FD:(n + 1) * FD],
                    start=True, stop=False,
                )
                for j in range(KT):
                    lhsT = mt_tiles[j][:, m * P:(m + 1) * P]
                    nc.tensor.matmul(
                        ops[:], lhsT, a_tiles[j][:, n * FD:(n + 1) * FD],
                        start=False, stop=(j == KT - 1),
                    )
                    if n == 0:
                        nc.tensor.matmul(
                            deg_ps[:], lhsT, ones_t[:],
                            start=(j == 0), stop=(j == KT - 1),
                        )
                if n == 0:
                    sca = emit_deg_scale(m, deg_ps)
                emit_epilogue(m, n, ops, sca)
    nc.compile()
    return nc


def _get_nc():
    if "nc" not in _CACHE:
        _CACHE["nc"] = _build_bass()
    return _CACHE["nc"]


def _make_in_maps(A_f32):
    import ml_dtypes

    bf = ml_dtypes.bfloat16
    Ab = A_f32.astype(bf)
    ATb = np.ascontiguousarray(Ab.T)

    ones = np.ones((P, 1), dtype=bf)
    iq = (0.25 * np.eye(P, dtype=np.float32)).astype(bf)
    in_maps = []
    for c in range(NCORES):
        sl = slice(c * R, (c + 1) * R)
        in_maps.append({
            "a": Ab,
            "art": np.ascontiguousarray(ATb[:, sl]),
            "ar": np.ascontiguousarray(Ab[sl, :]),
            "ones": ones,
            "iq": iq,
        })
    return in_maps


def kernel(A, w1a=None, w1b=None, w2a=None, **_unused):
    # w1a/w1b/w2a only enter the reference through a softmax over a
    # singleton axis (== 1.0), so the output does not depend on them.
    from concourse.bass_utils import run_bass_kernel_spmd

    A = np.asarray(A, dtype=np.float32)
    assert A.shape == (N, N), A.shape
    nc = _get_nc()
    in_maps = _make_in_maps(A)
    res = run_bass_kernel_spmd(nc, in_maps, core_ids=list(range(NCORES)))
    out = np.concatenate(
        [res.results[c]["out"] for c in range(NCORES)], axis=0
    )
    return out[None].astype(np.float32)



# revision 4
# speedup vs baseline: 1.5696x; 1.5696x over previous
"""Trainium2 Bass kernel for nn_GTN_72679436583060 (GTN message passing).

Math: with w-softmax over a singleton axis each GTConv is exactly 2*A, so

    out = 2 * rownorm(4*A@A + I) @ A
        = diag(64 / (16*rowsum(M1) + 1)) @ (M1@Ah + 0.0625*Ah_rows)

with Ah = A/2 (so M1 = Ah@Ah = (A@A)/4 ~ 128 fits fp8e4m3 range) and
rowsum/deg folded into a per-row reciprocal scale.

Sharding: row-wise over 8 cores, Ah replicated. Per core (rows R = 256):
  GEMM1 (transposed):  MT = Ah^T @ (Ah_rows^T)      (2048 x 256)
  deg:                 rowsum(M1) via ones-column matmuls on MT
  GEMM2:               P = M1 @ Ah + 0.0625*Ah_rows  (256 x 2048)
  epilogue:            out = P * (64 / (16*deg1 + 1)) per-row, bf16 out

All matmuls in fp8e4m3 with perf_mode=DoubleRow (K=256 per instruction,
two k-slabs per 3D access pattern [128, 2, f]), fp32 PSUM accumulation.
GEMM1 runs k-pair-outer so the PE tracks the streaming A DMA; warmup
matmuls during the initial DMA window ramp the PE HAM clock. Output is
DMA'd as bf16 and upcast on the host to halve the output tail.
"""

import numpy as np

N = 2048
P = 128
NCORES = 8
R = N // NCORES        # 256 rows per core
KT = N // P            # 16 partition tiles
KP = KT // 2           # 8 k-pair tiles (DoubleRow)
MT = R // P            # 2 row subtiles per core
FD = 512               # PSUM bank free dim (fp32)
NT2 = N // FD          # 4 GEMM2 n-tiles

_CACHE = {}


def _build_bass():
    from contextlib import ExitStack

    import concourse.bass as bass  # noqa: F401
    import concourse.mybir as mybir
    import concourse.tile as tile
    from concourse import bacc

    dt = mybir.dt
    fp32 = dt.float32
    bf16 = dt.bfloat16
    fp8 = dt.float8e4
    Alu = mybir.AluOpType
    DR = mybir.MatmulPerfMode.DoubleRow

    nc = bacc.Bacc(None, target_bir_lowering=False)
    a_d = nc.dram_tensor("a", [N, N], fp8, kind="ExternalInput")
    art_d = nc.dram_tensor("art", [P, KT, R], fp8, kind="ExternalInput")
    ar_d = nc.dram_tensor("ar", [R, N], fp8, kind="ExternalInput")
    ones_d = nc.dram_tensor("ones", [P, 1], fp8, kind="ExternalInput")
    iq_d = nc.dram_tensor("iq", [P, P], fp8, kind="ExternalInput")
    out_d = nc.dram_tensor("out", [R, N], bf16, kind="ExternalOutput")

    with tile.TileContext(nc) as tc, ExitStack() as ctx:
        a_pool = ctx.enter_context(tc.tile_pool(name="a", bufs=KP))
        art_pool = ctx.enter_context(tc.tile_pool(name="art", bufs=KP))
        ar_pool = ctx.enter_context(tc.tile_pool(name="ar", bufs=1))
        mt_pool = ctx.enter_context(tc.tile_pool(name="mt", bufs=KP))
        const_pool = ctx.enter_context(tc.tile_pool(name="const", bufs=1))
        outsb_pool = ctx.enter_context(tc.tile_pool(name="outsb", bufs=4))
        sc_pool = ctx.enter_context(tc.tile_pool(name="sc", bufs=4))

        zeros_t = const_pool.tile([P, FD], bf16, tag="zeros")
        nc.vector.memset(zeros_t[:], 0.0)

        # Stream A k-slab pairs (and matching ART pairs) in k order; they
        # stay resident: GEMM1 uses A slabs as lhsT, GEMM2 reuses them as
        # rhs. AR/ones/iq are only needed from GEMM2 on, so they go last.
        # Big A slabs alternate between the two HWDGE queues.
        a_tiles, art_tiles = [], []
        for t in range(KP):
            rt = art_pool.tile([P, 2, R], fp8, tag="art")
            nc.sync.dma_start(rt[:], art_d[:, 2 * t:2 * t + 2, :])
            art_tiles.append(rt)
            at = a_pool.tile([P, 2, N], fp8, tag="a")
            nc.sync.dma_start(at[:, 0, :], a_d[2 * t * P:(2 * t + 1) * P, :])
            nc.scalar.dma_start(
                at[:, 1, :], a_d[(2 * t + 1) * P:(2 * t + 2) * P, :]
            )
            a_tiles.append(at)
        ar_t = ar_pool.tile([P, 2, N], fp8, tag="ar")
        nc.scalar.dma_start(ar_t[:, 0, :], ar_d[0:P, :])
        nc.scalar.dma_start(ar_t[:, 1, :], ar_d[P:R, :])
        ones_t = const_pool.tile([P, 1], fp8, tag="ones")
        nc.sync.dma_start(ones_t[:], ones_d[:, :])
        iq_t = const_pool.tile([P, P], fp8, tag="iq")
        nc.sync.dma_start(iq_t[:], iq_d[:, :])

        # ---- GEMM1: MT[j, r] = sum_k Ah[k, j] * Ah_rows[r, k] ----
        # k-pair outer (DoubleRow contracts 256 rows per matmul). Two
        # j-groups share each PSUM bank: the even j's first matmul carries
        # start=True (resets has_written for the whole bank), the odd j's
        # first matmul then overwrites its all-unwritten half; later
        # matmuls accumulate. The PE executes its queue in program order,
        # so the start=True matmul always lands first. Warmup matmuls on
        # bank 7 keep the PE busy during the initial DMA window so the HAM
        # clock ramps (bank 7's real start=True matmul discards them).
        with tc.tile_pool(name="psum", bufs=8, space="PSUM") as psum_pool:
            pairs = []
            for b in range(KP):
                ps = psum_pool.tile([P, FD], fp32, tag="bank", name=f"pair_{b}")
                pairs.append(ps)
            for w in range(8):
                nc.tensor.matmul(
                    pairs[7][:], zeros_t[:, 0:P], zeros_t[:],
                    start=(w == 0), stop=False, skip_group_check=True,
                )
            for t in range(KP):
                for j in range(KT):
                    half = (j % 2) * R
                    nc.tensor.matmul(
                        pairs[j // 2][:, half:half + R],
                        a_tiles[t][:, :, j * P:(j + 1) * P],
                        art_tiles[t][:],
                        start=(t == 0 and j % 2 == 0), stop=(t == KP - 1),
                        perf_mode=DR, skip_group_check=True,
                    )
            # PSUM -> SBUF fp8 quantization of MT (values ~128 < 240 max)
            mt_tiles = [None] * KP
            for j in range(KT):
                half = (j % 2) * R
                if j % 2 == 0:
                    mt_tiles[j // 2] = mt_pool.tile(
                        [P, 2, R], fp8, tag="mt", name=f"mt_{j // 2}"
                    )
                nc.vector.tensor_copy(
                    mt_tiles[j // 2][:, j % 2, :],
                    pairs[j // 2][:, half:half + R],
                )

            # ---- GEMM2 + deg + epilogue ----
            # The 0.0625*I seed matmul doubles as each bank's accumulation
            # starter (start=True clears the bank and seeds 0.0625*Ah_rows).
            # m=0 runs jp-outer (tracks the mt copies); m=1 runs n-outer so
            # its banks finish staggered and the epilogues pipeline with PE.
            def emit_epilogue(m, n, psum_tile, sca):
                ot = outsb_pool.tile([P, FD], bf16, tag="ot",
                                     name=f"ot_{m}_{n}")
                nc.vector.tensor_scalar(
                    out=ot[:], in0=psum_tile[:], scalar1=sca[:],
                    scalar2=None, op0=Alu.mult,
                )
                eng = nc.sync if n % 2 == 0 else nc.scalar
                eng.dma_start(
                    out_d[m * P:(m + 1) * P, n * FD:(n + 1) * FD], ot[:]
                )

            def emit_deg_scale(m, deg_ps):
                # scale = 64 / (16*deg1 + 1) == 1 / (0.25*deg1 + 0.015625)
                t1 = sc_pool.tile([P, 1], fp32, tag="t1", name=f"t1_{m}")
                nc.vector.tensor_scalar(
                    out=t1[:], in0=deg_ps[:], scalar1=0.25, scalar2=0.015625,
                    op0=Alu.mult, op1=Alu.add,
                )
                sca = sc_pool.tile([P, 1], fp32, tag="sca", name=f"sca_{m}")
                nc.vector.reciprocal(sca[:], t1[:])
                return sca

            # m = 0: jp-outer
            m = 0
            outs_ps = [psum_pool.tile([P, FD], fp32, tag="bank",
                                      name=f"outps0_{i}") for i in range(NT2)]
            deg_full = psum_pool.tile([P, FD], fp32, tag="bank", name="deg_0")
            deg_ps = deg_full[:, 0:1]
            for n in range(NT2):
                nc.tensor.matmul(
                    outs_ps[n][:], iq_t[:],
                    ar_t[:, m, n * FD:(n + 1) * FD],
                    start=True, stop=False, skip_group_check=True,
                )
            for t in range(KP):
                lhsT = mt_tiles[t][:, :, m * P:(m + 1) * P]
                for n in range(NT2):
                    nc.tensor.matmul(
                        outs_ps[n][:], lhsT,
                        a_tiles[t][:, :, n * FD:(n + 1) * FD],
                        start=False, stop=(t == KP - 1),
                        perf_mode=DR, skip_group_check=True,
                    )
                for i in range(2):
                    nc.tensor.matmul(
                        deg_ps[:], mt_tiles[t][:, i, m * P:(m + 1) * P],
                        ones_t[:],
                        start=(t == 0 and i == 0), stop=(t == KP - 1 and i == 1),
                    )
            sca = emit_deg_scale(m, deg_ps)
            for n in range(NT2):
                emit_epilogue(m, n, outs_ps[n], sca)

            # m = 1: n-outer, deg rides along with the n=0 bank
            m = 1
            deg_full = psum_pool.tile([P, FD], fp32, tag="bank", name="deg_1")
            deg_ps = deg_full[:, 0:1]
            sca = None
            for n in range(NT2):
                ops = psum_pool.tile([P, FD], fp32, tag="bank",
                                     name=f"outps1_{n}")
                nc.tensor.matmul(
                    ops[:], iq_t[:], ar_t[:, m, n * FD:(n + 1) * FD],
                    start=True, stop=False, skip_group_check=True,
                )
                for t in range(KP):
                    nc.tensor.matmul(
                        ops[:], mt_tiles[t][:, :, m * P:(m + 1) * P],
                        a_tiles[t][:, :, n * FD:(n + 1) * FD],
                        start=False, stop=(t == KP - 1),
                        perf_mode=DR, skip_group_check=True,
                    )
                    if n == 0:
                        for i in range(2):
                            nc.tensor.matmul(
                                deg_ps[:],
                                mt_tiles[t][:, i, m * P:(m + 1) * P],
                                ones_t[:],
                                start=(t == 0 and i == 0),
                                stop=(t == KP - 1 and i == 1),
                            )
                if n == 0:
                    sca = emit_deg_scale(m, deg_ps)
                emit_epilogue(m, n, ops, sca)
    nc.compile()
    return nc


def _get_nc():
    if "nc" not in _CACHE:
        _CACHE["nc"] = _build_bass()
    return _CACHE["nc"]


def _make_in_maps(A_f32):
    import ml_dtypes

    f8 = ml_dtypes.float8_e4m3
    Ah = (A_f32 * 0.5).astype(f8)
    ATh = np.ascontiguousarray(Ah.T)

    ones = np.ones((P, 1), dtype=f8)
    iq = (0.0625 * np.eye(P, dtype=np.float32)).astype(f8)
    in_maps = []
    for c in range(NCORES):
        sl = slice(c * R, (c + 1) * R)
        art = np.ascontiguousarray(
            ATh[:, sl].reshape(KT, P, R).transpose(1, 0, 2)
        )
        in_maps.append({
            "a": Ah,
            "art": art,
            "ar": np.ascontiguousarray(Ah[sl, :]),
            "ones": ones,
            "iq": iq,
        })
    return in_maps


def kernel(A, w1a=None, w1b=None, w2a=None, **_unused):
    # w1a/w1b/w2a only enter the reference through a softmax over a
    # singleton axis (== 1.0), so the output does not depend on them.
    from concourse.bass_utils import run_bass_kernel_spmd

    A = np.asarray(A, dtype=np.float32)
    assert A.shape == (N, N), A.shape
    nc = _get_nc()
    in_maps = _make_in_maps(A)
    res = run_bass_kernel_spmd(nc, in_maps, core_ids=list(range(NCORES)))
    out = np.concatenate(
        [res.results[c]["out"] for c in range(NCORES)], axis=0
    )
    return out[None].astype(np.float32)
